# revision 14
# baseline (speedup 1.0000x reference)
"""Symmetric batch-hard triplet loss on 8 Trainium2 NeuronCores.

The distance matrix is symmetric, so only the upper-triangle block pairs
are computed: s_ij = 2 h_i.h_j - ||h_i||^2 - ||h_j||^2 = -d_ij rides a
fp8 DoubleRow GEMM whose last SIX contraction slots carry e4m3 norm
expansions for BOTH sides (B-side norms against an A-side constant, and
vice versa), making s symmetric.  Each computed [128, 512] tile is mined
two ways:

  rows  : DVE Max8 over the PSUM f32 tile -> top-8 of s per anchor row
          (2nd-largest s = 2nd-smallest distance after host merge).
  cols  : Act converts the tile to f16, the PE transposes it (vs. an f16
          identity) into half-bank PSUM tiles [128 cols, 512 row-vals],
          and DVE Max8 mines those -> top-8 per column row.

Same-label tiles take a 5th -BIG one-hot matmul; hardest positives come
from Act-copied + DVE row minima (gmin) and, for the class straddling a
core boundary, from per-column minima of the transposed strip (colmin).

Every core runs the SAME program over 9 units u = [1024 rows x 512 cols]:
[plain-off, diag0, diag1, off0(mask+colmin), off2..off6]; the host packs
per-core slab tensors (AST/MOV/A5/B5) so one SPMD program serves all
assignments, then merges all per-span top-8/gmin/colmin candidates into
exact per-row hardest positives/negatives and the masked mean.
"""

import functools

import numpy as np
import ml_dtypes

import concourse.bacc as bacc
import concourse.tile as tile
from concourse import mybir
from concourse.bass_utils import run_bass_kernel_spmd

FP8E4 = mybir.dt.float8e4
FP8E5 = mybir.dt.float8e5
F32 = mybir.dt.float32
F16 = mybir.dt.float16
BF16 = mybir.dt.bfloat16
E4 = ml_dtypes.float8_e4m3
E5 = ml_dtypes.float8_e5m2

N, D, C = 8192, 1024, 128
NCORES = 8
P = 128
JB = 512
NBLK = N // NCORES          # 1024 rows per block
MCH = NBLK // P             # 8 m-chunks per block
NJ = N // JB                # 16 column units
ALPHA = 0.1
EPS = 1e-7
BIG = 8192.0
NNORM = 3                   # e4m3 norm slots per side (6 total)
NSCALE = 8.0
NPOS = 9                    # u positions per core
NOFF = 7                    # off-diagonal (transposed) positions
KH = D // P                 # 8 k-tiles
KQ = KH // 2                # 4 DoubleRow matmuls per [128,512] tile
MODE = "fp8"

# Processing order: position -> role.  proc 0 is a plain off-u (mask
# operands get DMA slack), diag pairs next (their masks hit windows 1-2),
# off0 (the possibly-straddling masked unit) at proc 3.
ROLE_DIAG0, ROLE_DIAG1, ROLE_OFF = 0, 1, 2
PROC_ROLES = ["off", "off", "off", "diag0", "diag1", "off0", "off", "off",
              "off"]
OFF0_PROC = 5


def a_slab(proc, m):
    if proc in (3, 4):
        return m                          # diag: own block
    if proc == OFF0_PROC:
        return 8 + m
    if proc in (0, 1, 2, 6):
        return 16 + m                     # the 4-chunk block
    if proc == 7:
        return 24 + m
    return 32 + m                         # proc 8


def _row_slot(proc, m):
    return (proc * MCH + m) * 8


ROW_W = NPOS * MCH * 8                       # 576
COL_BASE = ROW_W


def _col_slot(coff, c4, g):
    return COL_BASE + ((coff * 4 + c4) * 2 + g) * 8


COL_W = NOFF * 4 * 2 * 8                     # 448
GMIN_BASE = COL_BASE + COL_W                 # 1024


def build_program(dmu, mode=MODE, warm=104):
    """dmu: sorted tuple of (du in {0,1}, m) diag tiles that get the -BIG
    mask matmul + gmin (union over cores, so the program is SPMD)."""
    n_gmin = len(dmu) + 1                    # +1: off0 m7
    cmin_base = GMIN_BASE + n_gmin
    W = cmin_base + 8

    nc = bacc.Bacc("TRN2", target_bir_lowering=False)
    # 40 stationary slabs: [0:8] own block (diag+...), [8:16] off0 block,
    # [16:24] 4-chunk block (procs 0,4,5,6), [24:32] proc-7 block,
    # [32:40] proc-8 block.  Host duplicates content as needed.
    AST = nc.dram_tensor("AST", [P, 40, KH, P], FP8E4, kind="ExternalInput")
    MOV = nc.dram_tensor("MOV", [P, NPOS, KH, JB], FP8E4,
                         kind="ExternalInput")
    A5 = nc.dram_tensor("A5", [P, 2, 9 * P], FP8E5, kind="ExternalInput")
    B5 = nc.dram_tensor("B5", [P, 2, 3 * JB], FP8E5, kind="ExternalInput")
    IDN = nc.dram_tensor("IDN", [P, P], F16, kind="ExternalInput")
    OUT = nc.dram_tensor("OUT", [P, W], F32, kind="ExternalOutput")

    with tile.TileContext(nc) as tc:
        with (
            tc.tile_pool(name="apool", bufs=1) as apool,
            tc.tile_pool(name="bpool", bufs=4) as bpool,
            tc.tile_pool(name="psum", bufs=3, space="PSUM") as pp,
            tc.tile_pool(name="tpp", bufs=5, space="PSUM") as tpp,
            tc.tile_pool(name="mpool", bufs=1) as mpool,
            tc.tile_pool(name="cpool", bufs=4) as cpool,
            tc.tile_pool(name="scpool", bufs=10) as scpool,
        ):
            wsrc = apool.tile([P, 2 * P], BF16, tag="wsrc")
            nc.vector.memset(wsrc[:], 0.0)
            wps = pp.tile([P, JB], F32, name="ps", tag="ps")
            for _ in range(warm):
                nc.tensor.matmul(wps[:, :P], wsrc[:, :P], wsrc[:, P:],
                                 start=True, stop=True)

            # --- head loads ---------------------------------------------
            def load_mov(proc):
                b = bpool.tile([P, KH, JB], FP8E4, tag="mov", name="mov")
                nc.sync.dma_start(out=b[:], in_=MOV[:, proc])
                return [b[:, 2 * t:2 * t + 2, :] for t in range(KQ)]

            mov = [None] * NPOS
            ast = apool.tile([P, 40, KH, P], FP8E4, tag="ast")
            # Only the Sync DMA queue starts early (~8us); Scalar/GpSimd
            # queues deliver no data before ~30us.  So everything the
            # first three (unmasked off) windows touch rides Sync in
            # consumption order, and the masked windows' operands arrive
            # on the slow queues while those windows are still far away.
            idt = apool.tile([P, P], F16, tag="idt")
            nc.sync.dma_start(out=idt[:], in_=IDN[:])
            mov[0] = load_mov(0)
            nc.sync.dma_start(out=ast[:, 16:17], in_=AST[:, 16:17])
            nc.sync.dma_start(out=ast[:, 17:24], in_=AST[:, 17:24])
            nc.scalar.dma_start(out=ast[:, 0:2], in_=AST[:, 0:2])
            nc.scalar.dma_start(out=ast[:, 2:8], in_=AST[:, 2:8])
            nc.scalar.dma_start(out=ast[:, 8:16], in_=AST[:, 8:16])
            a5 = apool.tile([P, 2, 9 * P], FP8E5, tag="a5")
            nc.gpsimd.dma_start(out=a5[:], in_=A5[:])
            b5 = apool.tile([P, 2, 3 * JB], FP8E5, tag="b5")
            nc.gpsimd.dma_start(out=b5[:], in_=B5[:])
            nc.gpsimd.dma_start(out=ast[:, 24:40], in_=AST[:, 24:40])

            out_sb = mpool.tile([P, W], F32, tag="out_sb")
            gsc = []                          # deferred gmin reductions

            coff = -1
            for proc in range(NPOS):
                role = PROC_ROLES[proc]
                if mov[proc] is None:
                    mov[proc] = load_mov(proc)
                for ahead in (1, 2):
                    if proc + ahead < NPOS and mov[proc + ahead] is None:
                        mov[proc + ahead] = load_mov(proc + ahead)
                is_off = role in ("off", "off0")
                if is_off:
                    coff += 1
                    # One full PSUM bank per column chunk: [128 cols,
                    # 1024 row-values] f16, filled by 8x4 transposes.
                    tp = [tpp.tile([P, 2 * JB], F16, tag="tp", name="tp")
                          for _ in range(4)]

                for m in range(MCH):
                    ps = pp.tile([P, JB], F32, name="ps", tag="ps")
                    at = [ast[:, a_slab(proc, m), 2 * t:2 * t + 2, :]
                          for t in range(KQ)]
                    masked = ((role == "diag0" and (0, m) in dmu)
                              or (role == "diag1" and (1, m) in dmu)
                              or (role == "off0" and m == MCH - 1))
                    for t in range(KQ):
                        nc.tensor.matmul(
                            ps[:], at[t], mov[proc][t],
                            start=(t == 0),
                            stop=(t == KQ - 1 and not masked),
                            perf_mode=mybir.MatmulPerfMode.DoubleRow)
                    if masked:
                        if role == "off0":
                            a5ap = a5[:, :, 8 * P:9 * P]
                            b5ap = b5[:, :, 2 * JB:3 * JB]
                            gslot = GMIN_BASE + len(dmu)
                        else:
                            du = 0 if role == "diag0" else 1
                            a5ap = a5[:, :, m * P:(m + 1) * P]
                            b5ap = b5[:, :, du * JB:(du + 1) * JB]
                            gslot = GMIN_BASE + dmu.index((du, m))
                        nc.tensor.matmul(
                            ps[:], a5ap, b5ap, start=False, stop=True,
                            perf_mode=mybir.MatmulPerfMode.DoubleRow)
                        sc = scpool.tile([P, JB], F32, tag="sc", name="sc")
                        nc.scalar.copy(sc[:], ps[:])
                        gsc.append((sc, gslot))

                    rs = _row_slot(proc, m)
                    if is_off:
                        # f16 convert frees the PSUM tile early (the row
                        # Max8 reads the SBUF f16 copy instead), then 4 PE
                        # transposes into the column tiles.  At a group
                        # boundary the column Max8s go first so the tp
                        # banks recycle before the next unit's transposes.
                        s16 = cpool.tile([P, JB], F16, tag="s16",
                                         name="s16")
                        nc.scalar.copy(s16[:], ps[:])
                        mi = m * P
                        for c4 in range(4):
                            nc.tensor.transpose(
                                tp[c4][:, mi:mi + P],
                                s16[:, c4 * P:(c4 + 1) * P], idt[:])
                        if m == 3 or m == MCH - 1:
                            g = m // 4
                            gs = slice(g * JB, (g + 1) * JB)
                            for c4 in range(4):
                                cs = _col_slot(coff, c4, g)
                                nc.vector.max(out_sb[:, cs:cs + 8],
                                              tp[c4][:, gs])
                                if role == "off0":
                                    cm = cmin_base + c4 * 2 + g
                                    nc.vector.tensor_reduce(
                                        out_sb[:, cm:cm + 1],
                                        tp[c4][:, gs],
                                        axis=mybir.AxisListType.X,
                                        op=mybir.AluOpType.min)
                        nc.vector.max(out_sb[:, rs:rs + 8], s16[:])
                    else:
                        nc.vector.max(out_sb[:, rs:rs + 8], ps[:])

                    # Spread the deferred gmin minima over later windows.
                    if proc >= 2 and gsc:
                        sc, gslot = gsc.pop(0)
                        nc.vector.tensor_reduce(
                            out_sb[:, gslot:gslot + 1], sc[:],
                            axis=mybir.AxisListType.X,
                            op=mybir.AluOpType.min)

                if proc == NPOS - 2:
                    # Rows of procs 0..7 are final; ship them early.
                    hw = _row_slot(NPOS - 1, 0)
                    nc.scalar.dma_start(out=OUT[:, :hw],
                                        in_=out_sb[:, :hw])

            assert not gsc, "gmin reductions must drain"
            hw = _row_slot(NPOS - 1, 0)
            nc.sync.dma_start(out=OUT[:, hw:], in_=out_sb[:, hw:])

    nc.compile()
    return nc


def _split_e4(x, terms, scale):
    out = []
    r = x.astype(np.float64).copy()
    for _ in range(terms):
        s = (r / scale).astype(E4)
        out.append(s)
        r -= scale * s.astype(np.float64)
    return out


def _assign(labels):
    """Build the per-core unit assignment + union diag-mask set.

    Returns (cores, dmu, straddle) where cores[c] is a dict with:
      off: list of NOFF (r, jb) units in proc order [p0, off0, p4..p8]
           indexed 0..6 as coff order,
      blocks: (own, off0_blk, chunk4_blk, p7_blk, p8_blk)
    """
    # off-units (r, jb) with jb // 2 > r
    alloff = [(r, jb) for r in range(NCORES) for jb in range(NJ)
              if jb // 2 > r]
    straddle = [labels[NBLK * (b + 1) - 1] == labels[NBLK * (b + 1)]
                for b in range(NCORES - 1)]
    # off0 per core: the boundary unit (b, 2b+2); core 7 takes a leftover.
    off0 = {b: (b, 2 * b + 2) for b in range(NCORES - 1)}
    pool = [u for u in alloff if u not in set(off0.values())]
    # core 7's off0: take a row-0 unit (max supply)
    u7 = next(u for u in pool if u[0] == 0)
    off0[7] = u7
    pool.remove(u7)
    # eight 4-chunks (same row-block) for procs 0,4,5,6
    supply = {}
    for u in pool:
        supply.setdefault(u[0], []).append(u)
    chunks4 = []
    for _ in range(NCORES):
        r = max(supply, key=lambda k: len(supply[k]))
        assert len(supply[r]) >= 4, supply
        chunks4.append([supply[r].pop() for _ in range(4)])
    singles = [u for r in sorted(supply) for u in supply[r]]
    assert len(singles) == 2 * NCORES, len(singles)
    cores = []
    for c in range(NCORES):
        ch = chunks4[c]
        p7, p8 = singles[2 * c], singles[2 * c + 1]
        # coff order = proc order of off units: 0, 3, 4, 5, 6, 7, 8
        off = [ch[0], off0[c], ch[1], ch[2], ch[3], p7, p8]
        cores.append({
            "off": off,
            "blocks": (c, off0[c][0], ch[0][0], p7[0], p8[0]),
        })
    # union diag mask set: per core, per diag unit du, masked m-chunks
    dmu = set()
    for c in range(NCORES):
        lab = labels[c * NBLK:(c + 1) * NBLK]
        for m in range(MCH):
            cl = lab[m * P:(m + 1) * P]
            lo = np.searchsorted(lab, cl.min(), "left")
            hi = np.searchsorted(lab, cl.max(), "right")
            for j in range(int(lo) // JB, (int(hi) - 1) // JB + 1):
                dmu.add((j, m))
    return cores, tuple(sorted(dmu)), straddle


def _proc_units(units, own):
    """units = [ch0, off0u, ch1, ch2, ch3, p7, p8] -> per-proc (r, jb)."""
    return [units[0], units[2], units[3], (own, 2 * own),
            (own, 2 * own + 1), units[1], units[4], units[5], units[6]]


def make_inputs(H, labels):
    H = np.ascontiguousarray(np.asarray(H, dtype=np.float32))
    labels = np.asarray(labels).astype(np.int64).ravel()
    perm = np.argsort(labels, kind="stable")
    H = H[perm]
    labels = labels[perm]
    cores, dmu, straddle = _assign(labels)

    Hr = H.astype(E4)
    xn = np.einsum("ij,ij->i", Hr.astype(np.float64), Hr.astype(np.float64))
    xsp = _split_e4(xn, NNORM, NSCALE)

    # Global moving form [P, NJ, KH, JB]: data rows H (e4m3), then the
    # stolen last-k-tile slots: partitions 122-124 carry the B-norm split,
    # 125-127 the A-side constant -NSCALE.
    Bg = Hr.T.reshape(KH, P, N).transpose(1, 0, 2).copy()
    for t in range(NNORM):
        Bg[P - 2 * NNORM + t, KH - 1, :] = xsp[t]
        Bg[P - NNORM + t, KH - 1, :] = -NSCALE
    Bg = Bg.reshape(P, KH, NJ, JB).transpose(0, 2, 1, 3).copy()

    # Global stationary form [P, 64 m-chunks, KH, P]: data rows 2H, slots
    # 122-124 = -NSCALE, 125-127 = the A-norm split.
    Ag = ((2.0 * Hr.astype(np.float32)).T.astype(E4)
          .reshape(KH, P, N).transpose(1, 0, 2).copy())
    for t in range(NNORM):
        Ag[P - 2 * NNORM + t, KH - 1, :] = -NSCALE
        Ag[P - NNORM + t, KH - 1, :] = xsp[t]
    Ag = Ag.reshape(P, KH, N // P, P).transpose(0, 2, 1, 3).copy()

    oh = labels[None, :] == np.arange(C, dtype=np.int64)[:, None]  # [C, N]

    in_maps = []
    for c in range(NCORES):
        sp = cores[c]
        own, ob, cb, p7b, p8b = sp["blocks"]
        ast = np.zeros((P, 40, KH, P), dtype=E4)
        for i, blk in enumerate((own, ob, cb, p7b, p8b)):
            ast[:, 8 * i:8 * i + 8] = Ag[:, blk * MCH:(blk + 1) * MCH]
        mov = np.zeros((P, NPOS, KH, JB), dtype=E4)
        units = sp["off"]
        proc_units = _proc_units(units, own)
        for pidx, (_, jb) in enumerate(proc_units):
            mov[:, pidx] = Bg[:, jb]
        a5 = np.zeros((P, 2, 9 * P), dtype=E5)
        a5[:C, 0, :8 * P] = (-BIG * oh[:, own * NBLK:(own + 1) * NBLK]
                             ).astype(E5)
        ob_m7 = slice(ob * NBLK + 7 * P, ob * NBLK + 8 * P)
        a5[:C, 0, 8 * P:] = (-BIG * oh[:, ob_m7]).astype(E5)
        b5 = np.zeros((P, 2, 3 * JB), dtype=E5)
        b5[:C, 0, 0 * JB:1 * JB] = oh[:, 2 * own * JB:(2 * own + 1) * JB
                                      ].astype(E5)
        b5[:C, 0, 1 * JB:2 * JB] = oh[:, (2 * own + 1) * JB:
                                      (2 * own + 2) * JB].astype(E5)
        jb0 = units[1][1]
        b5[:C, 0, 2 * JB:3 * JB] = oh[:, jb0 * JB:(jb0 + 1) * JB].astype(E5)
        in_maps.append({"AST": ast, "MOV": mov, "A5": a5, "B5": b5,
                        "IDN": np.eye(P, dtype=np.float16)})
    return in_maps, cores, dmu, perm, labels


@functools.lru_cache(maxsize=2)
def _get_program(dmu):
    return build_program(dmu)


def _finalize(outs, cores, dmu, labels):
    """Merge per-core mining outputs into the masked-mean loss."""
    n_gmin = len(dmu) + 1
    cmin_base = GMIN_BASE + n_gmin
    # hn candidates per global (sorted) row
    cand = [[] for _ in range(0)]
    hn_c = np.full((N, 0), -np.inf, dtype=np.float32)
    row_lists = [[] for _ in range(N // P)]   # per 128-row chunk
    hp = np.full(N, -np.inf, dtype=np.float64)

    def add_rows(chunk, vals):                # vals [P, 8]
        row_lists[chunk].append(vals)

    for c in range(NCORES):
        o = np.asarray(outs[c], dtype=np.float32)
        sp = cores[c]
        own = sp["blocks"][0]
        units = sp["off"]
        proc_units = _proc_units(units, own)
        # row-direction top-8s
        for proc, (r, jb) in enumerate(proc_units):
            for m in range(MCH):
                rs = _row_slot(proc, m)
                add_rows(r * MCH + m, o[:, rs:rs + 8])
        # column-direction top-8s (off units only, in coff order)
        off_procs = [0, 1, 2, 5, 6, 7, 8]
        for coff, proc in enumerate(off_procs):
            r, jb = proc_units[proc]
            for c4 in range(4):
                chunk = (jb * JB + c4 * P) // P
                for g in range(2):
                    cs = _col_slot(coff, c4, g)
                    add_rows(chunk, o[:, cs:cs + 8])
        # hp: gmin slots (diag union + off0 m7)
        for k, (du, m) in enumerate(dmu):
            rows = slice(own * NBLK + m * P, own * NBLK + (m + 1) * P)
            v = -o[:, GMIN_BASE + k].astype(np.float64) - BIG
            np.maximum.at(hp, np.arange(rows.start, rows.stop), v)
        ob = sp["blocks"][1]
        rows = np.arange(ob * NBLK + 7 * P, ob * NBLK + 8 * P)
        v = -o[:, GMIN_BASE + len(dmu)].astype(np.float64) - BIG
        np.maximum.at(hp, rows, v)
        # hp: colmin slots (off0 columns)
        jb0 = units[1][1]
        for c4 in range(4):
            colrows = np.arange(jb0 * JB + c4 * P, jb0 * JB + (c4 + 1) * P)
            for g in range(2):
                v = -o[:, cmin_base + c4 * 2 + g].astype(np.float64) - BIG
                np.maximum.at(hp, colrows, v)

    # exact top-2 per row over all candidate spans
    hn = np.empty(N, dtype=np.float64)
    for ch in range(N // P):
        allc = np.concatenate(row_lists[ch], axis=1).astype(np.float64)
        part = np.partition(allc, allc.shape[1] - 2, axis=1)
        second = part[:, -2]
        hn[ch * P:(ch + 1) * P] = -second     # 2nd-smallest distance
    hp = np.maximum(hp, EPS)                  # dist clamp (matches ref)
    hn = np.maximum(hn, EPS)
    loss = np.maximum(hp - hn + ALPHA, 0.0)
    rel = loss > EPS
    cnt = rel.sum()
    if cnt == 0:
        return np.float32(0.0)
    return np.float32(loss[rel].sum() / cnt)


def kernel(H, labels):
    in_maps, cores, dmu, perm, slab = make_inputs(H, labels)
    res = run_bass_kernel_spmd(_get_program(dmu), in_maps,
                               list(range(NCORES)))
    return _finalize([r["OUT"] for r in res.results], cores, dmu, slab)
